# revision 52
# baseline (speedup 1.0000x reference)
"""BasicSSM Trainium2 kernel (bf16 datapath, host-transposed x, few big DMAs).

Math: A_bar = expm(delta*A); u = x @ (delta*B)^T; h_t = h_{t-1} @ A_bar^T + u_t;
y = h @ C^T.

Because A = 0.05*randn - 0.5*I (documented construction in the reference), the
spectral radius of P = A_bar^T is ~0.65-0.75, so P^d decays below the bf16
noise floor by d ~ 32.  The scan is computed as a windowed convolution
    H[s] = sum_{d=0}^{W-1} u[s-d] @ P^d          (W = 8*N_D0 lags, adaptive)
which makes sequence sharding communication-free (each core only needs a
W-row halo of x).

Sharding: 8 cores = 4 batches x 2 sequence halves (communication-free).
The whole device datapath is bf16 (x and y are transported as bf16; PSUM
accumulation stays fp32); end-to-end error ~4e-3 norm-relative vs the 2e-2
gate.

HW measurements (ablation benches on these cores) showed the kernel is
DMA-bound and that DMA transfers largely serialize regardless of queue, at
~250 GB/s/core for the 16x128-tile XBAR-transpose path vs ~330 GB/s/core
for plain loads, with ~2us of extra critical-path cost per additional DMA
instruction.  The design therefore minimizes DMA count and avoids the XBAR:

  x is shipped from the host already transposed (and bf16): xs[d, j] with
  j = halo+seq position (host layout prep, same class as the bbt/pc/ct
  weight prep).  Per iteration the device issues only 5 DMAs:
    1 x-load   (128,8,2112) bf16 - ONE plain full-speed load, halo included
    2 u8 builds (SBUF->SBUF, overlapping-AP 8-lag stacking, per 1024 cols)
    2 y stores (1024 rows each)

Per core stages:
  stage 1: per 512-col span, 8 accumulating matmuls against the resident
           x^T tile -> u^T master (16, 2112) (5 PSUM->SBUF copies)
  stage 2: per 1024-col window pair, ONE overlapping-AP SBUF->SBUF DMA
           builds the 8-lag-stacked tile u8[(m,d_rev), j] =
           u^T[m, base+d_rev+j]; N_D0 accumulating matmuls per 512-half
           against host-built P-power stacks -> H^T
  stage 3: y tile (128,512) = H^T_slice.T @ C^T (PSUM) -> bf16 copy ->
           ybuf; one 1024-row store per 8 tiles
Scheduling: iteration k's stage-3 (loop-carried htm of k-1) interleaves
into iteration k's stage-1 spans (measured faster than y-phase-first or
cross-iteration x prefetch, both of which lost time to the serialized DMA
stream).  SP carries the x load + u8 builds, ACT the y stores.  PSUM:
1 bank u, 1 bank h, 6-deep psy pool.  The For_i timing body holds 4
software-pipelined iterations (body-boundary cost amortized; deeper
bodies regress on instruction-fetch).
"""

import numpy as np
import ml_dtypes

BF16 = np.dtype(ml_dtypes.bfloat16)
FP8 = np.dtype(ml_dtypes.float8_e4m3)

D_MODEL = 1024
D_STATE = 16
BATCH = 4
SEQ = 4096
N_CORES = 8
HALF = SEQ // 2           # 2048 rows of output per core
HP = 64                   # halo rows (supports window up to 7*8 = 56 lags)
ROWS = HP + HALF          # 2112
NYT = HALF // 128         # 16 y-tiles
NW = HALF // 512          # 4 scan windows of 512
N_D0 = 4                  # 8-lag groups -> window W = 32 lags (adaptive,
                          # widened at run time if P decays slowly; HP=64
                          # supports N_D0 <= 7)
U8F = 512 + 8 * N_D0 - 1  # u8 tile free size
LM = 8 * N_D0 - 1         # left margin inside u8 tile
SPANS = [(0, 512), (512, 512), (1024, 512), (1536, 512), (2048, HP)]
PS_U_BUFS = 1
PS_H_BUFS = 1
PS_Y_BUFS = 6
U8_ON_POOL = False
Y_COPY_3ENG = False       # rotate stage-3 PSUM->SBUF copies over DVE/ACT/Pool
U8_NWIN = 2               # 512-col windows covered per u8 build (1, 2, or 4)
ST_NTILES = 8             # 128-row y tiles per store DMA (4, 8, or 16)
BODY_ITERS = 4            # logical iterations per For_i body (timing variant)
XSPAN = 2048              # rows per XBAR transpose DMA (512, 1024, or 2048)
HALO_MERGE = False        # fold the 64-row halo into one 2112-row transpose
                          # (requires XSPAN == 2048)
XTSP_BUFS = 0             # xtsp pool depth override (0 = 2 iterations worth)
# --- ablation knobs (timing experiments only; break correctness) ---
SKIP_XBAR = False
SKIP_U8DMA = False
SKIP_STORES = False
SKIP_ST = False
SKIP_WINMM = False
SKIP_YMM = False
XBAR_2Q = False           # split the big transpose across SP and ACT queues
STORE_2Q = False          # alternate y stores between ACT and SP queues
XBAR_PLAIN = False        # ablation: plain (non-transpose) x load, timing only
XHOST_T = True            # host ships x already transposed (D_MODEL, ROWS);
                          # device does ONE plain full-speed load, no XBAR
U8_2Q = False             # alternate u8 builds between SP and ACT queues
XS_SWIZZLED = False       # host ships xs already in device tile layout
                          # [128, 8, ROWS] -> single huge-descriptor load
YS_SWIZZLED = False       # ys left in device layout [128, NYT*D_MODEL];
                          # host un-permutes after gather
Y_FIRST = False           # emit the whole y-phase (prev htm) before stage-1
                          # so PE/DVE/ACT work overlaps the big x load
U8_ENG = "sync"           # queue for u8 builds: sync | scalar
STORE_ENG = "scalar"      # queue for y stores: scalar | sync
PREFETCH_X = False        # load iteration k+1's x tile during iteration k
                          # (stage-1 reads a resident tile, never a fresh DMA)
XLOAD_PIECES = 1          # split the hostT x load into N column pieces
N_D0_FORCE = 0            # override the adaptive window (0 = adaptive)
U8_EARLY = False          # split gate-span utm copies so u8 builds unblock
                          # after the first 64 cols
XFP8 = False              # ship x as fp8e4m3 (halves the dominant x load;
                          # stage-1 matmul runs bf16 lhsT x fp8 rhs)
Y_PRE = 0                 # y-phase quarters emitted before st(0) (0..4)
PSUM_UNIFIED = False      # one 8-deep PSUM ring shared by psu/psh/psy


def _set_window(n_d0):
    global N_D0, U8F, LM
    N_D0 = n_d0
    U8F = 512 + 8 * N_D0 - 1
    LM = 8 * N_D0 - 1

_CACHE = {}
LAST_RESULTS = None  # BassKernelResults from the most recent run (for profiling)
TRACE = False


def _expm(M):
    """Scaling-and-squaring Taylor expm in float64 (16x16, ||M|| ~ 0.7)."""
    M = np.asarray(M, dtype=np.float64)
    nrm = np.linalg.norm(M, 1)
    s = max(0, int(np.ceil(np.log2(max(nrm, 1e-300)))) + 1) if nrm > 0.5 else 0
    Ms = M / (2.0 ** s)
    E = np.eye(M.shape[0])
    T = np.eye(M.shape[0])
    for k in range(1, 40):
        T = T @ Ms / k
        E = E + T
    for _ in range(s):
        E = E @ E
    return E


def _build_program(loop_n=None):
    """Build the (shared, SPMD) Bass program.  loop_n=None: one-shot
    correctness program (external xs/ys).  loop_n=int: hardware-loop timing
    variant (For_i, body = BODY_ITERS software-pipelined logical iterations,
    xs/ys internal so dispatch cost is negligible).  loop_n="unrollN":
    straight-line N-iteration variant used by the timeline-sim harness."""

    import concourse.bass as bass
    import concourse.bacc as bacc
    import concourse.mybir as mybir
    import concourse.tile as tile

    f32 = mybir.dt.float32
    bf16 = mybir.dt.bfloat16
    xdt = mybir.dt.float8e4 if XFP8 else bf16
    nc = bacc.Bacc(
        "TRN2", target_bir_lowering=False, debug=False, num_devices=N_CORES
    )

    if XS_SWIZZLED:
        xs_shape = [128, 8 * ROWS]
    elif XHOST_T:
        xs_shape = [D_MODEL, ROWS]
    else:
        xs_shape = [ROWS, D_MODEL]
    ys_shape = [128, NYT * D_MODEL] if YS_SWIZZLED else [HALF, D_MODEL]
    if loop_n is None:
        xs = nc.dram_tensor("xs", xs_shape, xdt, kind="ExternalInput")
        ys = nc.dram_tensor("ys", ys_shape, bf16, kind="ExternalOutput")
    else:
        xs = nc.dram_tensor("xs", xs_shape, xdt)
        ys = nc.dram_tensor("ys", ys_shape, bf16)
        done = nc.dram_tensor("done", [128, 1], bf16, kind="ExternalOutput")
    bbt = nc.dram_tensor("bbt", [D_MODEL, D_STATE], bf16, kind="ExternalInput")
    pc = nc.dram_tensor("pc", [128, N_D0 * D_STATE], bf16, kind="ExternalInput")
    ct = nc.dram_tensor("ct", [D_STATE, D_MODEL], bf16, kind="ExternalInput")

    with tile.TileContext(nc) as tc:
        with (
            tc.tile_pool(name="consts", bufs=1) as consts,
            tc.tile_pool(name="masters", bufs=2) as masters,
            tc.tile_pool(name="xtsp",
                         bufs=XTSP_BUFS or 2 * (HALF // XSPAN)) as xtsp,
            tc.tile_pool(name="xhp", bufs=2) as xhp,
            tc.tile_pool(name="u8", bufs=2 * (NW // U8_NWIN)) as u8p,
            tc.tile_pool(name="yout", bufs=2) as youtp,
            tc.tile_pool(name="ps_u", bufs=(8 if PSUM_UNIFIED else PS_U_BUFS),
                         space=bass.MemorySpace.PSUM) as ps_u,
            tc.tile_pool(name="ps_h", bufs=PS_H_BUFS, space=bass.MemorySpace.PSUM) as ps_h,
            tc.tile_pool(name="ps_y", bufs=PS_Y_BUFS, space=bass.MemorySpace.PSUM) as ps_y,
        ):
            if PSUM_UNIFIED:
                # all PSUM tiles ride one 8-deep ring (tag "ps" in ps_u pool)
                ps_h = ps_u
                ps_y = ps_u
            # --- constants ---
            bbt_s = consts.tile([128, 8, D_STATE], bf16)  # (dpart, kchunk, n)
            nc.sync.dma_start(
                bbt_s[:], bbt[:].rearrange("(k p) n -> p k n", p=128)
            )
            pc_s = consts.tile([128, N_D0 * D_STATE], bf16)
            nc.gpsimd.dma_start(pc_s[:], pc[:])
            ct_s = consts.tile([D_STATE, D_MODEL], bf16)
            nc.gpsimd.dma_start(ct_s[:], ct[:])
            # warm the ACT activation-function table off the critical path
            warm = consts.tile([1, 2], bf16)
            nc.scalar.copy(warm[:, 1:2], warm[:, 0:1])

            state = {}

            # stage-1 load: XBAR DMA-transposes (16x128-tile crossbar path,
            # bf16-only) move x[r0:r0+rn, :] -> xtm[p, c, j] = x[r0+j, c*128+p].
            # XSPAN controls DMA granularity: bigger spans = fewer DMAs
            # (lower per-DMA latency overhead) but stage-1 consumption waits
            # on a bigger transfer.  The halo (64 rows) is always separate.
            def xbar_all():
                if XHOST_T:
                    if SKIP_XBAR:
                        nc.gpsimd.memset(state["xbig"][0][:, 0, 0:2], 0)
                        return
                    if XS_SWIZZLED:
                        nc.sync.dma_start(
                            state["xbig"][0][:],
                            xs[:].rearrange("p (c j) -> p c j", c=8),
                        )
                    elif XLOAD_PIECES == 2:
                        xsv = xs[:].rearrange("(c p) j -> p c j", p=128)
                        nc.sync.dma_start(
                            state["xbig"][0][:, :, :1024], xsv[:, :, :1024]
                        )
                        nc.sync.dma_start(
                            state["xbig"][0][:, :, 1024:], xsv[:, :, 1024:]
                        )
                    else:
                        nc.sync.dma_start(
                            state["xbig"][0][:],
                            xs[:].rearrange("(c p) j -> p c j", p=128),
                        )
                    return
                if SKIP_XBAR:
                    for t in state["xbig"]:
                        nc.gpsimd.memset(t[:, 0, 0:2], 0)
                    if state.get("xhalo") is not None:
                        nc.gpsimd.memset(state["xhalo"][:, 0, 0:2], 0)
                    return
                if HALO_MERGE:
                    nc.sync.dma_start_transpose(
                        state["xbig"][0][:], xs[0:ROWS, :]
                    )
                    return
                if XBAR_PLAIN:
                    # timing-only: same bytes, natural layout, big descriptors
                    for b in range(HALF // XSPAN):
                        eng = nc.scalar if (XBAR_2Q and b % 2) else nc.sync
                        eng.dma_start(
                            state["xbig"][b][:].rearrange(
                                "p a (k b) -> p (a k) b", b=1024
                            ),
                            xs[b * XSPAN:(b + 1) * XSPAN, :].rearrange(
                                "(k p) c -> p k c", p=128
                            ),
                        )
                    if state.get("xhalo") is not None:
                        nc.gpsimd.memset(state["xhalo"][:, 0, 0:2], 0)
                    return
                if XBAR_2Q and HALF // XSPAN >= 2:
                    for b in range(HALF // XSPAN):
                        eng = nc.scalar if b % 2 else nc.sync
                        eng.dma_start_transpose(
                            state["xbig"][b][:], xs[b * XSPAN:(b + 1) * XSPAN, :]
                        )
                elif XBAR_2Q:
                    # split the single big transpose into two half-row spans
                    h = XSPAN // 2
                    nc.sync.dma_start_transpose(
                        state["xbig"][0][:, :, :h], xs[0:h, :]
                    )
                    nc.scalar.dma_start_transpose(
                        state["xbig"][0][:, :, h:], xs[h:XSPAN, :]
                    )
                else:
                    for b in range(HALF // XSPAN):
                        nc.sync.dma_start_transpose(
                            state["xbig"][b][:], xs[b * XSPAN:(b + 1) * XSPAN, :]
                        )
                nc.sync.dma_start_transpose(
                    state["xhalo"][:], xs[HALF:HALF + HP, :]
                )

            def alloc_xtm():
                if XHOST_T:
                    xbm = xtsp.tile([128, 8, ROWS], xdt, tag="xtsp",
                                    name="xbm")
                    state["xbig"] = [xbm]
                    state["xhalo"] = None
                    return
                if HALO_MERGE:
                    assert XSPAN == HALF
                    xbm = xtsp.tile([128, 8, ROWS], bf16, tag="xtsp",
                                    name="xbm")
                    state["xbig"] = [xbm]
                    state["xhalo"] = None
                    return
                state["xbig"] = [
                    xtsp.tile([128, 8, XSPAN], bf16, tag="xtsp", name="xb")
                    for _ in range(HALF // XSPAN)
                ]
                state["xhalo"] = xhp.tile([128, 8, HP], bf16, tag="xhalo",
                                          name="xh")

            # stage-1 compute: u^T[:, r0:r0+rn] = Bb @ x[r0:r0+rn, :]^T
            def st(i):
                if SKIP_ST:
                    return
                r0, rn = SPANS[i]
                if HALO_MERGE or XHOST_T:
                    src = state["xbig"][0]
                    c0 = r0
                elif i == 4:
                    src = state["xhalo"]
                    c0 = 0
                else:
                    src = state["xbig"][r0 // XSPAN]
                    c0 = r0 % XSPAN
                psu = ps_u.tile([D_STATE, 512], f32,
                                tag="ps" if PSUM_UNIFIED else "psu")
                for cc in range(8):
                    nc.tensor.matmul(
                        psu[:, :rn],
                        bbt_s[:, cc, :],
                        src[:, cc, c0:c0 + rn],
                        start=(cc == 0),
                        stop=(cc == 7),
                    )
                gate = U8_EARLY and rn > 64 and any(
                    U8_NWIN * (g + 1) == i for g in range(NW // U8_NWIN))
                if gate:
                    nc.vector.tensor_copy(
                        state["utm"][:, r0:r0 + 64], psu[:, :64])
                    nc.vector.tensor_copy(
                        state["utm"][:, r0 + 64:r0 + rn], psu[:, 64:rn])
                else:
                    nc.vector.tensor_copy(
                        state["utm"][:, r0:r0 + rn], psu[:, :rn])

            # stage-2 u8 build g: the 8-lag stacked tile for windows
            # [g*U8_NWIN, (g+1)*U8_NWIN) (overlapping-AP DMA stacks 8 lags
            # into partitions; d reversed so the shift step is +1; reversal
            # baked into pc on the host).
            def u8build(g):
                w0 = HP + 512 * U8_NWIN * g
                u8f = LM + 512 * U8_NWIN
                utm = state["utm"]
                u8 = u8p.tile([128, u8f], bf16, tag="u8", name="u8t")
                state["u8"][g] = u8
                utm_base = utm[:, 0:1]
                src = bass.AP(
                    utm_base.tensor,
                    utm_base.offset + (w0 - LM - 7),
                    [[ROWS, D_STATE], [1, 8], [1, u8f]],
                )
                if SKIP_U8DMA:
                    nc.gpsimd.memset(u8[:, 0:2], 0)
                else:
                    (nc.gpsimd if U8_ON_POOL else nc.sync).dma_start(
                        u8[:], src)

            # stage-2 compute for one 512-col window: N_D0 accumulating
            # matmuls against the host-built P-power stacks
            def winmm(w):
                if SKIP_WINMM:
                    return
                g, h = divmod(w, U8_NWIN)
                u8 = state["u8"][g]
                psh = ps_h.tile([D_STATE, 512], f32,
                                tag="ps" if PSUM_UNIFIED else "psh")
                for d0 in range(N_D0):
                    off = LM - 8 * d0 + 512 * h
                    nc.tensor.matmul(
                        psh[:],
                        pc_s[:, d0 * D_STATE:(d0 + 1) * D_STATE],
                        u8[:, off:off + 512],
                        start=(d0 == 0),
                        stop=(d0 == N_D0 - 1),
                    )
                if w % 2 == 0:
                    nc.vector.tensor_copy(
                        state["htm"][:, w * 512:(w + 1) * 512], psh[:]
                    )
                else:
                    nc.scalar.copy(
                        state["htm"][:, w * 512:(w + 1) * 512], psh[:]
                    )

            # stage-3 quarter G: y-tiles 4G..4G+3 from src_htm into ybuf;
            # a store DMA fires whenever the next ST_NTILES tiles are done
            # (row t*128+p <- ybuf[p, t, :])
            def y4(G, src_htm, ybuf):
                if SKIP_YMM:
                    pass
                else:
                 for t4 in range(4):
                    t = 4 * G + t4
                    for g in range(2):
                        psy = ps_y.tile([128, 512], f32,
                                        tag="ps" if PSUM_UNIFIED else "psy")
                        nc.tensor.matmul(
                            psy[:],
                            src_htm[:, t * 128:(t + 1) * 128],
                            ct_s[:, g * 512:(g + 1) * 512],
                            start=True,
                            stop=True,
                        )
                        dst = ybuf[:, t, g * 512:(g + 1) * 512]
                        if Y_COPY_3ENG:
                            k = (8 * G + 2 * t4 + g) % 3
                            if k == 0:
                                nc.vector.tensor_copy(dst, psy[:])
                            elif k == 1:
                                nc.scalar.copy(dst, psy[:])
                            else:
                                nc.gpsimd.tensor_copy(dst, psy[:])
                        elif g == 0:
                            nc.vector.tensor_copy(dst, psy[:])
                        else:
                            nc.scalar.copy(dst, psy[:])
                if (4 * (G + 1)) % ST_NTILES == 0 and not SKIP_STORES:
                    c = (4 * (G + 1)) // ST_NTILES - 1
                    r0 = c * ST_NTILES * 128
                    if STORE_ENG == "sync":
                        seng = nc.sync
                    else:
                        seng = nc.sync if (STORE_2Q and c % 2) else nc.scalar
                    if YS_SWIZZLED:
                        w = ST_NTILES * D_MODEL
                        seng.dma_start(
                            ys[:, c * w:(c + 1) * w].rearrange(
                                "p (tt c) -> p tt c", tt=ST_NTILES
                            ),
                            ybuf[:, c * ST_NTILES:(c + 1) * ST_NTILES, :],
                        )
                    else:
                        seng.dma_start(
                            ys[r0:r0 + ST_NTILES * 128, :].rearrange(
                                "(tt p) c -> p tt c", p=128
                            ),
                            ybuf[:, c * ST_NTILES:(c + 1) * ST_NTILES, :],
                        )

            def alloc_ybuf():
                yb = youtp.tile([128, NYT, D_MODEL], bf16, tag="ybuf",
                                name="yb")
                if SKIP_YMM:
                    nc.gpsimd.memset(yb[:, 0, 0:2], 0)
                return yb

            # one logical iteration; y-phase of prev_htm is interleaved
            # into this iteration's stage-1 spans.  u8 build g issues as
            # soon as its source spans are in utm; window matmuls follow
            # one build behind.
            def prime_x():
                alloc_xtm()
                xbar_all()
                state["next_x"] = (state["xbig"], state["xhalo"])

            def schedule(prev_htm, flush, load_next=True):
                utm_t = masters.tile([D_STATE, ROWS], bf16, tag="utm", name="utm_t")
                state["utm"] = utm_t
                htm_t = masters.tile([D_STATE, HALF], bf16, tag="htm", name="htm_t")
                state["htm"] = htm_t
                if SKIP_ST:
                    nc.gpsimd.memset(utm_t[:, 0:2], 0)
                if SKIP_WINMM:
                    nc.gpsimd.memset(htm_t[:, 0:2], 0)
                nbuilds = NW // U8_NWIN
                state["u8"] = [None] * nbuilds
                if PREFETCH_X:
                    cur_x = state["next_x"]
                    if load_next:
                        alloc_xtm()
                        xbar_all()
                        state["next_x"] = (state["xbig"], state["xhalo"])
                    state["xbig"], state["xhalo"] = cur_x
                else:
                    alloc_xtm()
                    xbar_all()
                ybuf = alloc_ybuf() if (prev_htm is not None or flush) else None
                npre = 4 if Y_FIRST else Y_PRE
                if prev_htm is not None:
                    for G in range(npre):
                        y4(G, prev_htm, ybuf)
                wdone = 0
                for i in range(5):
                    st(i)
                    for g in range(nbuilds):
                        # build g reads utm cols up to HP+512*U8_NWIN*(g+1)-1,
                        # i.e. everything through span U8_NWIN*(g+1)
                        if U8_NWIN * (g + 1) == i:
                            u8build(g)
                            while wdone < U8_NWIN * (g + 1):
                                winmm(wdone)
                                wdone += 1
                    if prev_htm is not None and npre + i < 4:
                        y4(npre + i, prev_htm, ybuf)
                while wdone < NW:
                    winmm(wdone)
                    wdone += 1
                cur = state["htm"]
                if flush:
                    for G in range(4):
                        y4(G, cur, ybuf)
                return cur

            if loop_n is None:
                if PREFETCH_X:
                    prime_x()
                schedule(None, flush=True, load_next=False)
            elif isinstance(loop_n, str) and loop_n.startswith("unroll"):
                n = int(loop_n[6:])
                if PREFETCH_X:
                    prime_x()
                prev = None
                for k in range(n):
                    prev = schedule(prev, flush=(k == n - 1),
                                    load_next=(k < n - 1))
                nc.sync.dma_start(done[:], pc_s[:, 0:1])
            else:
                # hardware-loop timing variant: body = 2 logical iterations
                # (pool rotation consistent across the loop boundary);
                # htm pre-allocated so the body's first y-phase has a
                # loop-carried source (garbage data on the first pass —
                # timing only, ys is never read)
                prev = masters.tile([D_STATE, HALF], bf16, tag="htm")
                nc.gpsimd.memset(prev[:], 0)
                if PREFETCH_X:
                    prime_x()
                with tc.For_i(0, loop_n, 1):
                    for _ in range(BODY_ITERS):
                        prev = schedule(prev, flush=False)
                nc.sync.dma_start(done[:], pc_s[:, 0:1])

    nc.compile()
    return nc


def _get_runner(nc):
    """Cached shard_map runner (mirrors bass2jax.run_bass_via_pjrt but the
    jitted callable persists across kernel() calls)."""
    import jax
    import numpy as _np
    from jax.sharding import Mesh, PartitionSpec
    try:
        from jax.experimental.shard_map import shard_map
    except ImportError:
        from jax.shard_map import shard_map
    import concourse.mybir as mybir
    from concourse import bass2jax

    bass2jax.install_neuronx_cc_hook()
    part_name = nc.partition_id_tensor.name if nc.partition_id_tensor else None
    in_names, out_names, out_avals, zero_outs = [], [], [], []
    for alloc in nc.m.functions[0].allocations:
        if not isinstance(alloc, mybir.MemoryLocationSet):
            continue
        name = alloc.memorylocations[0].name
        if alloc.kind == "ExternalInput":
            if name != part_name:
                in_names.append(name)
        elif alloc.kind == "ExternalOutput":
            shape = tuple(alloc.tensor_shape)
            dtype = mybir.dt.np(alloc.dtype)
            out_names.append(name)
            out_avals.append(jax.core.ShapedArray(shape, dtype))
            zero_outs.append(_np.zeros(shape, dtype))
    n_params = len(in_names)
    n_outs = len(out_avals)
    all_names = in_names + out_names
    if part_name is not None:
        all_names = all_names + [part_name]
    donate = tuple(range(n_params, n_params + n_outs))

    def _body(*args):
        operands = list(args)
        if part_name is not None:
            operands.append(bass2jax.partition_id_tensor())
        outs = bass2jax._bass_exec_p.bind(
            *operands,
            out_avals=tuple(out_avals),
            in_names=tuple(all_names),
            out_names=tuple(out_names),
            lowering_input_output_aliases=(),
            sim_require_finite=True,
            sim_require_nnan=True,
            nc=nc,
        )
        return tuple(outs)

    devices = jax.devices()[:N_CORES]
    mesh = Mesh(np.asarray(devices), ("core",))
    specs = (PartitionSpec("core"),) * (n_params + n_outs)
    sharded = jax.jit(
        shard_map(_body, mesh=mesh, in_specs=specs,
                  out_specs=(PartitionSpec("core"),) * n_outs, check_rep=False),
        donate_argnums=donate, keep_unused=True,
    )
    return sharded, in_names, out_names, zero_outs


def _run_spmd_cached(nc, in_maps):
    import jax
    if "runner" not in _CACHE:
        _CACHE["runner"] = _get_runner(nc)
    sharded, in_names, out_names, zero_outs = _CACHE["runner"]
    concat_in = [
        np.concatenate([np.asarray(in_maps[c][n]) for c in range(N_CORES)], axis=0)
        for n in in_names
    ]
    concat_zero = [np.concatenate([z] * N_CORES, axis=0) for z in zero_outs]
    outs = sharded(*concat_in, *concat_zero)
    outs = [np.asarray(o) for o in outs]
    results = []
    for c in range(N_CORES):
        m = {}
        for i, n in enumerate(out_names):
            per = outs[i].shape[0] // N_CORES
            m[n] = outs[i][c * per:(c + 1) * per]
        results.append(m)
    return results


def bench_hw(x, A, B, C, delta, n=2048, n0=1024):
    """Absolute HW timing via a For_i-looped variant of the program with
    internal xs/ys (tiny external I/O).  Returns (times, per_iter_seconds)."""
    import time as _time
    import jax
    kernel(x, A, B, C, delta)  # fills _CACHE["last_in_maps"]
    in_maps = _CACHE["last_in_maps"]

    results = {}
    for n_iter in (n0, n):
        key = f"loopnc_{n_iter}"
        if key not in _CACHE:
            _CACHE[key] = _build_program(loop_n=n_iter)
            _CACHE[key + "_runner"] = _get_runner(_CACHE[key])
        ncl = _CACHE[key]
        sharded, in_names, out_names, zero_outs = _CACHE[key + "_runner"]
        concat_in = [
            np.concatenate(
                [np.asarray(in_maps[c][nm]) for c in range(N_CORES)], axis=0
            )
            for nm in in_names
        ]
        best = 1e9
        for rep in range(14):
            concat_zero = [np.concatenate([z] * N_CORES, axis=0) for z in zero_outs]
            t0 = _time.time()
            r = sharded(*concat_in, *concat_zero)
            jax.block_until_ready(r)
            dt = _time.time() - t0
            if rep > 0:
                best = min(best, dt)
        results[n_iter] = best
    per_iter = (results[n] - results[n0]) / (n - n0) / BODY_ITERS
    return results, per_iter


def kernel(x, A, B, C, delta):
    global LAST_RESULTS
    from concourse.bass_utils import run_bass_kernel_spmd

    x = np.ascontiguousarray(np.asarray(x, dtype=np.float32))
    dl = float(np.asarray(delta).reshape(-1)[0])

    # host-side tiny-weight prep (float64)
    A_bar = _expm(dl * np.asarray(A, np.float64))       # (N, N)
    P = A_bar.T
    pows = [np.eye(D_STATE)]
    for _ in range(8 * 7):
        pows.append(pows[-1] @ P)
    # widen the window if P^(8*N_D0) hasn't decayed below bf16 significance
    want = 4
    while want < 7 and np.linalg.norm(pows[8 * want], 2) > 2e-4:
        want += 1
    if N_D0_FORCE:
        want = N_D0_FORCE
    if want != N_D0:
        _set_window(want)
        _CACHE.clear()
    # u8 partition layout is (m, d_rev) = m*8 + d_rev (partition-major DMA
    # legality) with d reversed so the shift step is +1; pc rows match:
    # pc[m*8 + d_rev, d0*16 + n] = P^(8*d0 + 7 - d_rev)[m, n]
    pc_np = np.zeros((128, N_D0 * D_STATE), np.float32)
    for d0 in range(N_D0):
        for dr in range(8):
            for m in range(D_STATE):
                pc_np[m * 8 + dr, d0 * D_STATE:(d0 + 1) * D_STATE] = \
                    pows[8 * d0 + 7 - dr][m].astype(np.float32)
    pc_np = pc_np.astype(BF16)
    bbt_np = np.ascontiguousarray(
        (dl * np.asarray(B, np.float64)).T.astype(np.float32)
    ).astype(BF16)
    ct_np = np.ascontiguousarray(np.asarray(C, np.float32).T).astype(BF16)

    if "nc" not in _CACHE:
        _CACHE["nc"] = _build_program()
    nc = _CACHE["nc"]
    assert np.linalg.norm(pows[8 * N_D0], 2) <= 5e-3, "window too short for this A"

    xbf = x.astype(BF16)
    in_maps = []
    for core in range(N_CORES):
        b, half = divmod(core, 2)
        t0 = half * HALF
        if XHOST_T:
            xdt_np = FP8 if XFP8 else BF16
            xs_np = np.zeros((D_MODEL, ROWS), xdt_np)
            if t0 >= HP:
                xs_np[:, :HP] = xbf[b, t0 - HP:t0].T.astype(xdt_np)
            xs_np[:, HP:] = xbf[b, t0:t0 + HALF].T.astype(xdt_np)
            if XS_SWIZZLED:
                xs_np = np.ascontiguousarray(
                    xs_np.reshape(8, 128, ROWS).transpose(1, 0, 2)
                ).reshape(128, 8 * ROWS)
        else:
            xs_np = np.zeros((ROWS, D_MODEL), BF16)
            if t0 >= HP:
                xs_np[:HP] = xbf[b, t0 - HP:t0]
            xs_np[HP:] = xbf[b, t0:t0 + HALF]
        in_maps.append({
            "xs": xs_np, "bbt": bbt_np, "pc": pc_np, "ct": ct_np,
        })

    _CACHE["last_in_maps"] = in_maps
    if TRACE:
        res = run_bass_kernel_spmd(nc, in_maps, list(range(N_CORES)), trace=True)
        LAST_RESULTS = res
        results = res.results
    else:
        results = _run_spmd_cached(nc, in_maps)

    y = np.empty((BATCH, SEQ, D_MODEL), np.float32)
    for core in range(N_CORES):
        b, half = divmod(core, 2)
        ys_res = results[core]["ys"]
        if YS_SWIZZLED:
            ys_res = ys_res.reshape(128, NYT, D_MODEL).transpose(
                1, 0, 2).reshape(HALF, D_MODEL)
        y[b, half * HALF:(half + 1) * HALF, :] = ys_res.astype(np.float32)
    return y

